# revision 1
# baseline (speedup 1.0000x reference)
"""Trainium2 Bass kernel for nn_Decoder (LSTM decoder + dual attention).

Sharding: data-parallel over batch B=128 across 8 NeuronCores (16 samples each).
On-chip layouts are feature-major ("transposed": features on partitions, time x
batch on the free dim) so biases are per-partition activation biases, the LSTM
emits gates directly in feature-major layout, and attention/projections run as
time-batched GEMMs with stationary weights.

Phases per core:
  P1: K/V projections (char+tag), X@Wih^T for all steps -> DRAM scratch.
  P2: sequential LSTM over T steps: 64 self-loading matmuls/step (Whh^T tiles
      stationary, h streaming), elementwise cell update staggered per E-chunk
      so the next step's matmuls overlap the current step's elementwise tail.
  P3: softmax attention + output projections as 128-step-block GEMMs.

Weights arrive pre-transposed / bf16 from the host (layout prep only); matmul
accumulation is fp32 in PSUM and the LSTM cell state stays fp32.
"""

import contextlib

import numpy as np
import ml_dtypes

B, T, E, G, NCH, SC, STG = 128, 256, 512, 2048, 128, 256, 32
NCORES = 8
PB = B // NCORES  # per-core batch = 16
EC = E // 128     # E chunks = 4

_cache = {}


def _build(Ts, reps=1):
    import concourse.mybir as mybir
    from concourse import bacc
    from concourse import masks
    from concourse.tile import TileContext

    dt = mybir.dt
    AF = mybir.ActivationFunctionType
    AX = mybir.AxisListType
    TB = min(128, Ts)            # P3 time-block size
    NBLK = Ts // TB
    SLAB = min(8, Ts)            # P2 xwt prefetch slab (steps)
    QE = float(1.0 / np.sqrt(E))

    nc = bacc.Bacc(None, dynamic_dma_scratch_size=4096)

    def din(name, shape, d=dt.bfloat16):
        return nc.dram_tensor(name, shape, d, kind="ExternalInput")

    ceT_d = din("ceT", [PB, E, SC])
    teT_d = din("teT", [PB, E, STG])
    xT_d = din("xT", [NCH, Ts, PB])
    whhT_d = din("whhT", [E, G])
    wihT_d = din("wihT", [NCH, G])
    PNAMES = ("wqcT", "wocT", "wqtT", "wotT")
    KNAMES = ("wkcT", "wvcT", "wktT", "wvtT")
    w_d = {nm: din(nm, [E, E]) for nm in PNAMES + KNAMES}
    outWT_d = din("outWT", [2 * E, NCH])
    gbias_d = din("gbias", [G], dt.float32)
    bias_d = {nm: din(nm, [E], dt.float32)
              for nm in ("bqc", "bvc", "boc", "bqt", "bvt", "bot")}
    outb_d = din("outb", [NCH], dt.float32)
    h0T_d = din("h0T", [E, PB])
    c0T_d = din("c0T", [E, PB], dt.float32)

    xwt_d = nc.dram_tensor("xwt", [Ts, EC, 4, 128, PB], dt.bfloat16)
    out_d = nc.dram_tensor("out", [PB, Ts, NCH], dt.float32, kind="ExternalOutput")

    with TileContext(nc) as tc, contextlib.ExitStack() as ctx:
        pp = ctx.enter_context(tc.tile_pool(name="persist", bufs=1))
        rep_cm = tc.For_i(0, reps, 1) if reps > 1 else None

        # ---- persistent tiles -------------------------------------------
        hT = pp.tile([128, EC, Ts, PB], dt.bfloat16)      # h after each step
        cT = pp.tile([128, EC, PB], dt.float32)
        h0T = pp.tile([128, EC, PB], dt.bfloat16)
        kcT = pp.tile([128, EC, PB, SC], dt.bfloat16)     # K_char^T per sample
        vc = pp.tile([128, 2, PB, E], dt.bfloat16)        # V_char [s,e] per sample
        ktT = pp.tile([128, EC, PB, STG], dt.bfloat16)
        vt = pp.tile([STG, PB, E], dt.bfloat16)           # V_tag, partitions 0..31
        wsb = {nm: pp.tile([128, EC, EC, 128], dt.bfloat16, name=nm)
               for nm in PNAMES}
        outWT = pp.tile([128, 2 * EC, NCH], dt.bfloat16)
        gbias = pp.tile([128, 16], dt.float32)
        bsb = {nm: pp.tile([128, EC], dt.float32, name=nm) for nm in bias_d}
        outb = pp.tile([128, 1], dt.float32)
        id_bf = pp.tile([128, 128], dt.bfloat16)
        id_f32 = pp.tile([128, 128], dt.float32)
        masks.make_identity(nc, id_bf[:, :])
        masks.make_identity(nc, id_f32[:, :])

        nc.sync.dma_start(h0T[:, :, :], h0T_d.rearrange("(k p) b -> p k b", p=128))
        nc.sync.dma_start(cT[:, :, :], c0T_d.rearrange("(k p) b -> p k b", p=128))
        nc.sync.dma_start(gbias[:, :], gbias_d.rearrange("(j p) -> p j", p=128))
        for nm in bias_d:
            nc.sync.dma_start(bsb[nm][:, :],
                              bias_d[nm].rearrange("(k p) -> p k", p=128))
        nc.sync.dma_start(outb[:, :], outb_d[:, None])
        for nm in PNAMES:
            for k in range(EC):
                nc.sync.dma_start(
                    wsb[nm][:, k, :, :],
                    w_d[nm][k * 128:(k + 1) * 128, :]
                    .rearrange("p (m c) -> p m c", c=128),
                )
        nc.sync.dma_start(outWT[:, :, :],
                          outWT_d.rearrange("(k p) n -> p k n", p=128))

        # =================================================================
        # P1: XWT GEMM -> DRAM scratch; K/V projections
        # =================================================================
        if rep_cm is not None:
            ctx.enter_context(rep_cm)
            # each repetition restarts from the initial cell state
            nc.sync.dma_start(cT[:, :, :],
                              c0T_d.rearrange("(k p) b -> p k b", p=128))
        with tc.tile_pool(name="p1", bufs=1) as p1, \
             tc.tile_pool(name="p1w", bufs=2) as p1w, \
             tc.tile_pool(name="ps1", bufs=3, space="PSUM") as ps1:
            wih = p1.tile([128, 16, 128], dt.bfloat16)
            nc.sync.dma_start(wih[:, :, :],
                              wihT_d.rearrange("p (j c) -> p j c", c=128))
            wkv = {nm: p1.tile([128, EC, EC, 128], dt.bfloat16, name=nm)
                   for nm in KNAMES}
            for nm in KNAMES:
                for k in range(EC):
                    nc.sync.dma_start(
                        wkv[nm][:, k, :, :],
                        w_d[nm][k * 128:(k + 1) * 128, :]
                        .rearrange("p (m c) -> p m c", c=128),
                    )
            ceT = p1.tile([128, EC, PB, SC], dt.bfloat16)
            teT = p1.tile([128, EC, PB, STG], dt.bfloat16)
            for k in range(EC):
                nc.sync.dma_start(
                    ceT[:, k, :, :],
                    ceT_d[:, k * 128:(k + 1) * 128, :].rearrange("i p s -> p i s"),
                )
                nc.sync.dma_start(
                    teT[:, k, :, :],
                    teT_d[:, k * 128:(k + 1) * 128, :].rearrange("i p s -> p i s"),
                )

            # --- XWT = Wih^T @ x, bias folded, spilled to DRAM ---
            ncc = max(1, Ts * PB // 512)
            tpc = Ts // ncc
            for cc in range(ncc):
                xbuf = p1w.tile([128, tpc, PB], dt.bfloat16, tag="xbuf")
                nc.sync.dma_start(xbuf[:, :, :],
                                  xT_d[:, cc * tpc:(cc + 1) * tpc, :])
                for j in range(16):
                    ps = ps1.tile([128, tpc, PB], dt.float32, tag="ps")
                    nc.tensor.matmul(ps[:, :, :], wih[:, j, :], xbuf[:, :, :])
                    stg = p1w.tile([128, tpc, PB], dt.bfloat16, tag="stg",
                                   bufs=4)
                    if j % 2 == 0:
                        nc.vector.tensor_scalar_add(stg[:, :, :], ps[:, :, :],
                                                    gbias[:, j:j + 1])
                    else:
                        nc.scalar.activation(stg[:, :, :], ps[:, :, :],
                                             AF.Identity,
                                             bias=gbias[:, j:j + 1])
                    nc.sync.dma_start(
                        xwt_d[cc * tpc:(cc + 1) * tpc, j % 4, j // 4, :, :]
                        .rearrange("t p b -> p t b"),
                        stg[:, :, :],
                    )

            # --- kcT[m, i, s] ---
            for m in range(EC):
                for i2 in range(0, PB, 2):
                    ps = ps1.tile([128, 2, SC], dt.float32, tag="ps")
                    for k in range(EC):
                        nc.tensor.matmul(
                            ps[:, :, :], wkv["wkcT"][:, k, m, :],
                            ceT[:, k, i2:i2 + 2, :],
                            start=(k == 0), stop=(k == EC - 1),
                        )
                    nc.any.tensor_copy(kcT[:, m, i2:i2 + 2, :], ps[:, :, :])
            # --- vc[sc, i, e] ---
            for i in range(PB):
                for sc in range(SC // 128):
                    ps = ps1.tile([128, E], dt.float32, tag="ps")
                    for k in range(EC):
                        nc.tensor.matmul(
                            ps[:, :], ceT[:, k, i, sc * 128:(sc + 1) * 128],
                            wkv["wvcT"][:, k, :, :].rearrange("p m c -> p (m c)"),
                            start=(k == 0), stop=(k == EC - 1),
                        )
                    nc.any.tensor_copy(vc[:, sc, i, :], ps[:, :])
            # --- ktT[m, i, s] ---
            for m in range(EC):
                ps = ps1.tile([128, PB, STG], dt.float32, tag="ps")
                for k in range(EC):
                    nc.tensor.matmul(
                        ps[:, :, :], wkv["wktT"][:, k, m, :], teT[:, k, :, :],
                        start=(k == 0), stop=(k == EC - 1),
                    )
                nc.any.tensor_copy(ktT[:, m, :, :], ps[:, :, :])
            # --- vt[s, i, e] (partitions 0..31) ---
            for i in range(PB):
                ps = ps1.tile([STG, E], dt.float32, tag="ps")
                for k in range(EC):
                    nc.tensor.matmul(
                        ps[:, :], teT[:, k, i, :],
                        wkv["wvtT"][:, k, :, :].rearrange("p m c -> p (m c)"),
                        start=(k == 0), stop=(k == EC - 1),
                    )
                nc.any.tensor_copy(vt[:, i, :], ps[:, :])

        # =================================================================
        # P2: sequential LSTM.  gate-tile j = gt*4 + ec, gt in (i,f,g,o)
        # =================================================================
        with tc.tile_pool(name="p2", bufs=1) as p2p, \
             tc.tile_pool(name="p2w", bufs=3) as p2, \
             tc.tile_pool(name="p2s", bufs=2) as p2s, \
             contextlib.ExitStack() as ctx2:
            gps = [ctx2.enter_context(
                tc.tile_pool(name=f"g{e}", bufs=1, space="PSUM"))
                for e in range(EC)]
            whh = p2p.tile([128, EC, 16, 128], dt.bfloat16)
            for k in range(EC):
                nc.sync.dma_start(
                    whh[:, k, :, :],
                    whhT_d[k * 128:(k + 1) * 128, :]
                    .rearrange("p (j c) -> p j c", c=128),
                )
            for t in range(Ts):
                if t % SLAB == 0:
                    slab = p2s.tile([128, SLAB, EC, 4, PB], dt.bfloat16, tag="slab")
                    for e in range(EC):
                        for g in range(4):
                            nc.sync.dma_start(
                                slab[:, :, e, g, :],
                                xwt_d[t:t + SLAB, e, g, :, :]
                                .rearrange("t p b -> p t b"),
                            )
                # gate order is (i, f, o, g) after the host-side permute, so
                # one sigmoid covers i|f|o of all E-chunks and one tanh all g
                ga = p2.tile([128, EC, 4, PB], dt.float32, tag="ga")
                for ec in range(EC):
                    gp = gps[ec].tile([128, 4, PB], dt.float32, tag=f"gt{ec}")
                    for k in range(EC):
                        rhs = h0T[:, k, :] if t == 0 else hT[:, k, t - 1, :]
                        for gt in range(4):
                            nc.tensor.matmul(
                                gp[:, gt, :], whh[:, k, gt * 4 + ec, :], rhs,
                                start=(k == 0 and gt == 0),
                                stop=(k == EC - 1 and gt == 3),
                            )
                    nc.vector.tensor_add(ga[:, ec, :, :], gp[:, :, :],
                                         slab[:, t % SLAB, ec, :, :])
                sio = p2.tile([128, EC, 3, PB], dt.float32, tag="sio")
                tg = p2.tile([128, EC, PB], dt.float32, tag="tg")
                nc.scalar.activation(sio[:, :, :, :], ga[:, :, 0:3, :], AF.Sigmoid)
                nc.scalar.activation(tg[:, :, :], ga[:, :, 3, :], AF.Tanh)
                v = p2.tile([128, EC, PB], dt.float32, tag="v")
                u = p2.tile([128, EC, PB], dt.float32, tag="u")
                nc.vector.tensor_mul(v[:, :, :], sio[:, :, 1, :], cT[:, :, :])
                nc.vector.tensor_mul(u[:, :, :], sio[:, :, 0, :], tg[:, :, :])
                nc.vector.tensor_add(cT[:, :, :], u[:, :, :], v[:, :, :])
                tcc = p2.tile([128, EC, PB], dt.float32, tag="tcc")
                nc.scalar.activation(tcc[:, :, :], cT[:, :, :], AF.Tanh)
                nc.vector.tensor_mul(hT[:, :, t, :], sio[:, :, 2, :], tcc[:, :, :])

        # =================================================================
        # P3: attention + projections, per time block
        # =================================================================
        with tc.tile_pool(name="p3", bufs=1) as p3, \
             tc.tile_pool(name="p3w", bufs=2) as p3w, \
             tc.tile_pool(name="ps3", bufs=6, space="PSUM") as ps3:
            ncols = TB * PB
            nch = max(1, ncols // 512)
            cw = ncols // nch

            def proj(dst, wname, t0, bias, scale):
                # dst[:, m, trange, :] = scale*(W^T @ hT-block) + bias
                tw = cw // PB
                for m in range(EC):
                    for cc in range(nch):
                        ps = ps3.tile([128, tw, PB], dt.float32, tag="ps")
                        for k in range(EC):
                            nc.tensor.matmul(
                                ps[:, :, :], wsb[wname][:, k, m, :],
                                hT[:, k, t0 + cc * tw:t0 + (cc + 1) * tw, :],
                                start=(k == 0), stop=(k == EC - 1),
                            )
                        nc.vector.tensor_scalar(
                            dst[:, m, cc * tw:(cc + 1) * tw, :], ps[:, :, :],
                            scale, bias[:, m:m + 1],
                            op0=mybir.AluOpType.mult, op1=mybir.AluOpType.add,
                        )

            def oproj(dst, wname, src, bias):
                # dst[:, m, (i t)] = relu(W^T @ src + bias)
                for m in range(EC):
                    for cc in range(nch):
                        ps = ps3.tile([128, cw], dt.float32, tag="ps")
                        for k in range(EC):
                            nc.tensor.matmul(
                                ps[:, :], wsb[wname][:, k, m, :],
                                src[:, k, :, :].rearrange("p i t -> p (i t)")
                                [:, cc * cw:(cc + 1) * cw],
                                start=(k == 0), stop=(k == EC - 1),
                            )
                        nc.vector.tensor_scalar(
                            dst[:, m, :, :].rearrange("p i t -> p (i t)")
                            [:, cc * cw:(cc + 1) * cw],
                            ps[:, :], bias[:, m:m + 1], 0.0,
                            op0=mybir.AluOpType.add, op1=mybir.AluOpType.max,
                        )

            for blk in range(NBLK):
                t0 = blk * TB
                # ---- char attention ----
                qT = p3.tile([128, EC, TB, PB], dt.bfloat16, tag="qT")
                proj(qT, "wqcT", t0, bsb["bqc"], QE)
                ctxT = p3.tile([128, EC, PB, TB], dt.bfloat16, tag="ctxT")
                for i in range(PB):
                    pc = ps3.tile([128, SC], dt.float32, tag="ps")
                    for k in range(EC):
                        nc.tensor.matmul(
                            pc[:TB, :], qT[:, k, :, i], kcT[:, k, i, :],
                            start=(k == 0), stop=(k == EC - 1),
                        )
                    pe = p3w.tile([128, SC], dt.bfloat16, tag="pe", bufs=1)
                    dsum = p3w.tile([128, 1], dt.float32, tag="dsum")
                    nc.scalar.activation(pe[:TB, :], pc[:TB, :], AF.Exp,
                                         accum_out=dsum[:TB, :])
                    drec = p3w.tile([128, 1], dt.float32, tag="drec")
                    nc.vector.reciprocal(drec[:TB, :], dsum[:TB, :])
                    pn = p3w.tile([128, SC], dt.bfloat16, tag="pn", bufs=1)
                    nc.vector.tensor_scalar_mul(pn[:TB, :], pe[:TB, :],
                                                drec[:TB, 0:1])
                    pTt = p3w.tile([128, 2, 128], dt.bfloat16, tag="pTt")
                    for sc in range(2):
                        tp = ps3.tile([128, 128], dt.bfloat16, tag="ps")
                        nc.tensor.transpose(
                            tp[:, :TB], pn[:TB, sc * 128:(sc + 1) * 128],
                            id_bf[:TB, :TB],
                        )
                        nc.vector.tensor_copy(pTt[:, sc, :TB], tp[:, :TB])
                    cps = ps3.tile([128, EC, 128], dt.float32, tag="ps")
                    for m in range(EC):
                        for sc in range(2):
                            nc.tensor.matmul(
                                cps[:, m, :TB],
                                vc[:, sc, i, m * 128:(m + 1) * 128],
                                pTt[:, sc, :TB],
                                start=(m == 0 and sc == 0),
                                stop=(m == EC - 1 and sc == 1),
                            )
                    for m in range(EC):
                        nc.vector.tensor_scalar_add(
                            ctxT[:, m, i, :], cps[:, m, :TB],
                            bsb["bvc"][:, m:m + 1],
                        )
                orc = p3.tile([128, EC, PB, TB], dt.bfloat16, tag="orc")
                oproj(orc, "wocT", ctxT, bsb["boc"])

                # ---- tag attention ----
                qT2 = p3.tile([128, EC, TB, PB], dt.bfloat16, tag="qT")
                proj(qT2, "wqtT", t0, bsb["bqt"], QE)
                ptp = ps3.tile([128, PB, STG], dt.float32, tag="ps")
                for i in range(PB):
                    for k in range(EC):
                        nc.tensor.matmul(
                            ptp[:TB, i, :], qT2[:, k, :, i], ktT[:, k, i, :],
                            start=(i == 0 and k == 0),
                            stop=(i == PB - 1 and k == EC - 1),
                        )
                pte = p3w.tile([128, PB, STG], dt.bfloat16, tag="pte", bufs=1)
                nc.scalar.activation(pte[:TB, :, :], ptp[:TB, :, :], AF.Exp)
                tsum = p3w.tile([128, PB], dt.float32, tag="tsum")
                nc.vector.reduce_sum(tsum[:TB, :], pte[:TB, :, :], axis=AX.X)
                trec = p3w.tile([128, PB], dt.float32, tag="trec")
                nc.vector.reciprocal(trec[:TB, :], tsum[:TB, :])
                ptn = p3w.tile([128, PB, STG], dt.bfloat16, tag="ptn", bufs=1)
                ptT = p3w.tile([STG, PB, TB], dt.bfloat16, tag="ptT", bufs=1)
                for i in range(PB):
                    nc.vector.tensor_scalar_mul(ptn[:TB, i, :], pte[:TB, i, :],
                                                trec[:TB, i:i + 1])
                    tp2 = ps3.tile([STG, 128], dt.bfloat16, tag="ps")
                    nc.tensor.transpose(tp2[:, :TB], ptn[:TB, i, :],
                                        id_bf[:TB, :TB])
                    nc.vector.tensor_copy(ptT[:, i, :], tp2[:, :TB])
                ctxT2 = p3.tile([128, EC, PB, TB], dt.bfloat16, tag="ctxT")
                for i in range(PB):
                    cps = ps3.tile([128, EC, 128], dt.float32, tag="ps")
                    for m in range(EC):
                        nc.tensor.matmul(
                            cps[:, m, :TB], vt[:, i, m * 128:(m + 1) * 128],
                            ptT[:, i, :],
                            start=(m == 0), stop=(m == EC - 1),
                        )
                    for m in range(EC):
                        nc.vector.tensor_scalar_add(
                            ctxT2[:, m, i, :], cps[:, m, :TB],
                            bsb["bvt"][:, m:m + 1],
                        )
                ort = p3.tile([128, EC, PB, TB], dt.bfloat16, tag="ort")
                oproj(ort, "wotT", ctxT2, bsb["bot"])

                # ---- output projection + per-sample transpose ----
                for cc in range(nch):
                    ps = ps3.tile([128, cw], dt.float32, tag="ps")
                    for k in range(2 * EC):
                        src = orc if k < EC else ort
                        nc.tensor.matmul(
                            ps[:, :], outWT[:, k, :],
                            src[:, k % EC, :, :].rearrange("p i t -> p (i t)")
                            [:, cc * cw:(cc + 1) * cw],
                            start=(k == 0), stop=(k == 2 * EC - 1),
                        )
                    of = p3w.tile([128, cw], dt.float32, tag="of")
                    nc.vector.tensor_scalar_add(of[:, :], ps[:, :],
                                                outb[:, 0:1])
                    ns = cw // TB
                    for si in range(ns):
                        i = cc * ns + si
                        tps = ps3.tile([128, 128], dt.float32, tag="ps")
                        nc.tensor.transpose(
                            tps[:TB, :], of[:, si * TB:(si + 1) * TB],
                            id_f32[:, :],
                        )
                        oseg = p3w.tile([TB, NCH], dt.float32, tag="oseg")
                        nc.vector.tensor_copy(oseg[:, :], tps[:TB, :])
                        nc.sync.dma_start(out_d[i, t0:t0 + TB, :], oseg[:, :])

    nc.compile()
    return nc


# gate-row permutation: torch order (i,f,g,o) -> kernel order (i,f,o,g)
_GPERM = np.r_[0:E, E:2 * E, 3 * E:4 * E, 2 * E:3 * E]


def _prep_core(inputs, core, Ts=T):
    bf = ml_dtypes.bfloat16
    s = slice(core * PB, (core + 1) * PB)
    ce = inputs["char_encoding"][s]
    te = inputs["tag_encoding"][s]
    tos = inputs["true_output_seq"][s][:, :Ts]
    xs = np.concatenate(
        [np.zeros((PB, 1, NCH), np.float32), tos[:, 1:, :]], axis=1
    )
    m = {
        "ceT": np.ascontiguousarray(ce.transpose(0, 2, 1)).astype(bf),
        "teT": np.ascontiguousarray(te.transpose(0, 2, 1)).astype(bf),
        "xT": np.ascontiguousarray(xs.transpose(2, 1, 0)).astype(bf),
        "whhT": np.ascontiguousarray(inputs["lstm_Whh"][_GPERM].T).astype(bf),
        "wihT": np.ascontiguousarray(inputs["lstm_Wih"][_GPERM].T).astype(bf),
        "wqcT": np.ascontiguousarray(inputs["ca_Wq"].T).astype(bf),
        "wkcT": np.ascontiguousarray(inputs["ca_Wk"].T).astype(bf),
        "wvcT": np.ascontiguousarray(inputs["ca_Wv"].T).astype(bf),
        "wocT": np.ascontiguousarray(inputs["ca_Wo"].T).astype(bf),
        "wqtT": np.ascontiguousarray(inputs["ta_Wq"].T).astype(bf),
        "wktT": np.ascontiguousarray(inputs["ta_Wk"].T).astype(bf),
        "wvtT": np.ascontiguousarray(inputs["ta_Wv"].T).astype(bf),
        "wotT": np.ascontiguousarray(inputs["ta_Wo"].T).astype(bf),
        "outWT": np.ascontiguousarray(inputs["out_W"].T).astype(bf),
        "gbias": (inputs["lstm_bih"] + inputs["lstm_bhh"])[_GPERM]
        .astype(np.float32),
        "bqc": (inputs["ca_bq"] / np.sqrt(E)).astype(np.float32),
        "bvc": inputs["ca_bv"].astype(np.float32),
        "boc": inputs["ca_bo"].astype(np.float32),
        "bqt": (inputs["ta_bq"] / np.sqrt(E)).astype(np.float32),
        "bvt": inputs["ta_bv"].astype(np.float32),
        "bot": inputs["ta_bo"].astype(np.float32),
        "outb": inputs["out_b"].astype(np.float32),
        "h0T": np.ascontiguousarray(
            np.concatenate([inputs["char_hn"][0][s],
                            inputs["char_hn"][1][s]], -1).T).astype(bf),
        "c0T": np.ascontiguousarray(
            np.concatenate([inputs["char_cn"][0][s],
                            inputs["char_cn"][1][s]], -1).T).astype(np.float32),
    }
    return m


def kernel(**inputs):
    from concourse.bass_utils import run_bass_kernel_spmd

    inputs = {k: np.asarray(v, dtype=np.float32) for k, v in inputs.items()}
    if "nc" not in _cache:
        _cache["nc"] = _build(T)
    nc = _cache["nc"]
    in_maps = [_prep_core(inputs, c) for c in range(NCORES)]
    res = run_bass_kernel_spmd(nc, in_maps, list(range(NCORES)))
    _cache["last_res"] = res
    outs = [np.asarray(res.results[c]["out"]) for c in range(NCORES)]
    return np.concatenate(outs, axis=0).astype(np.float32)

